# revision 26
# baseline (speedup 1.0000x reference)
"""HNet energy-via-edge-matching kernel for 8 Trainium2 NeuronCores.

Math (matches the reference exactly, in exact integer arithmetic):
  temp[i,e] = 2*na[i, idx0[e]] + na[i, idx1[e]]          in {0,1,2,3}
  es = code[temp], code = [NOR=2, NCONV=3, NIMPL=5, AND=9]
  filter keeps es values in edge_type_filter, else NULL=0
  energies[i,j] = #{e: L[j,e]==es'[i,e] or L[j,e]==0}
               = null_count[j] + sum_{v kept} (temp==tmap[v]) . (L==v)
  output = energies - min(energies)

Device decomposition per core (4 point-groups x 2 cmp-groups):
  phase 1: tT[e,i] = sum_n S[n,e]*naT[n,i], S = 2*onehot(idx0)+onehot(idx1)
           -> fp8 DoubleRow matmuls.  Edges are permuted on the host so
           each 128-edge chunk touches ~2 node blocks (sorted by the
           unordered node-block pair of its endpoints); each chunk then
           needs only ceil(nblocks/2) DR matmuls instead of 4.
           A_v[e,i] = (tT==tmap[v]) masks (fp8), split across ACT/DVE.
  phase 2: per cmp tile nt (512 cols): B_v[e,j] = (LT==v) masks (fp8),
           energies[i,j] = sum_e A_v^T B_v via fp8 DoubleRow matmuls.
           Emission is pipelined at e-quarter granularity: phase-1 work
           for quarter k is emitted inside the nt=0 wave so the PE never
           waits for the full mask set (chains mid-stall on range deps).
Host only: input staging/layout (incl. the edge permutation), null_count
  row add, global min subtract during unshard (exact fp32 integer math).
"""

import numpy as np
import ml_dtypes

import concourse.bacc as bacc
import concourse.mybir as mybir
from concourse.tile import TileContext
from concourse.bass_utils import run_bass_kernel_spmd

# ---- problem constants (hardcoded from spec) ----
N_PTS, N_NODES, N_EDGES, N_CMP = 2048, 1024, 8192, 4096
PGROUPS, CGROUPS = 4, 2          # 8 cores = 4 point-groups x 2 cmp-groups
P = N_PTS // PGROUPS             # 512 points per core
C = N_CMP // CGROUPS             # 2048 cmp columns per core
ECHUNKS = N_EDGES // 128         # 64 edge chunks of 128
NKC = N_NODES // 128             # 8 node chunks of 128
NTILES = C // 512                # 4 cmp tiles of 512 per core
MTILES = P // 128                # 4 point chunks of 128 per core
QTRS = 4                         # e-quarters for pipelined emission
ECQ = ECHUNKS // QTRS            # 16 edge chunks per quarter
EBLK = 8                         # edge chunks per LT DMA block
EBQ = ECQ // EBLK                # LT blocks per quarter (2)

FP8 = mybir.dt.float8e4
F32 = mybir.dt.float32
NP_FP8 = ml_dtypes.float8_e4m3
DR = mybir.MatmulPerfMode.DoubleRow
EQ = mybir.AluOpType.is_equal
RELU = mybir.ActivationFunctionType.Relu

_CODE2TEMP = {2: 0, 3: 1, 5: 2, 9: 3}   # EDG code value -> temp index

_nc_cache: dict = {}


def _act_able(value, alphabet_max):
    return value == alphabet_max or value == 0


def _mask_op(nc, out, in_, value, alphabet_max, engine, bias_ap):
    """Emit out = (in_ == value) as {0.0, 1.0} fp8.

    "act" uses an exact one-relu indicator (valid when value is the
    alphabet max: relu(x-(value-1)); or value==0: relu(1-x)).
    "dve" uses is_equal.  Exact on these small-integer alphabets.
    """
    if engine == "act":
        if value == alphabet_max and value != 0:
            nc.scalar.activation(out, in_, RELU, bias=bias_ap(1 - value),
                                 scale=1.0)
        elif value == 0:
            nc.scalar.activation(out, in_, RELU, bias=bias_ap(1), scale=-1.0)
        else:
            raise ValueError(f"no act indicator for {value}")
        return
    if engine == "gps":
        nc.gpsimd.tensor_scalar(out=out, in0=in_, scalar1=float(value),
                                scalar2=None, op0=EQ)
        return
    nc.vector.tensor_scalar(out=out, in0=in_, scalar1=float(value),
                            scalar2=None, op0=EQ)


def _build_nc(pairs, plan_key):
    """Build the SPMD Bass program.

    pairs = tuple of (temp_val, L_val).
    plan_key = tuple per edge chunk of ((pA, pB), ...) block pairs.
    """
    nc = bacc.Bacc(None)
    plan = plan_key
    offs = []            # S slot offset per chunk
    tot2 = 0
    for mms in plan:
        offs.append(tot2)
        tot2 += 2 * len(mms)
    # quarter slice boundaries in S slots
    qoff = [offs[q * ECQ] for q in range(QTRS)] + [tot2]

    npair = len(pairs)
    tmax = max((tv for tv, _ in pairs), default=0)
    lmax = 9  # EDG alphabet max
    # pre-tiled inputs (host lays out so every DMA is per-partition dense):
    #   naT : [128, NKC*P]            [ki, ko*P+p]   = na[pg*P+p, ko*128+ki]
    #   S   : [128, TOT2*128]  slot s=(off_c+2j+h): [ki, s*128+el] =
    #           S[blk(c,j,h)*128+ki, perm_e(c*128+el)]
    #   LT  : [NTILES, QTRS, 128, ECQ*512] [nt,qt,ki, c*512+j] =
    #           L[cg*C+nt*512+j, ((qt*ECQ+c)*128+ki th permuted edge)]
    naT = nc.dram_tensor("naT", [128, NKC * P], FP8, kind="ExternalInput")
    S = nc.dram_tensor("S", [128, tot2 * 128], FP8, kind="ExternalInput")
    LT = nc.dram_tensor("LT", [NTILES, QTRS, 128, ECQ * 512], FP8,
                        kind="ExternalInput")
    en = nc.dram_tensor("en", [P, C], F32, kind="ExternalOutput")

    with TileContext(nc) as tc:
        with (
            tc.tile_pool(name="const", bufs=1) as const_pool,
            tc.tile_pool(name="amask", bufs=1) as a_pool,
            tc.tile_pool(name="bmask", bufs=3 * npair + 1) as b_pool,
            tc.tile_pool(name="lt", bufs=5) as lt_pool,
            tc.tile_pool(name="out", bufs=4) as out_pool,
            tc.tile_pool(name="chain", bufs=6, space="PSUM") as chain_pool,
            tc.tile_pool(name="tp", bufs=2, space="PSUM") as tp_pool,
        ):
            na_sb = const_pool.tile([128, NKC, P], FP8, tag="na")
            # low node blocks first: the first (key-sorted) phase-1 chunks
            # only touch them, so the first matmul isn't gated on all of na
            nc.sync.dma_start(out=na_sb[:, :NKC // 2, :],
                              in_=naT[:, :NKC // 2 * P])
            s_sb = const_pool.tile([128, tot2, 128], FP8, tag="s")

            def emit_s_dma(lo, hi):
                if hi > lo:
                    nc.sync.dma_start(
                        out=s_sb[:, lo:hi, :],
                        in_=S[:, lo * 128:hi * 128])

            # First quarter's S arrives in three slices up front so the very
            # first phase-1 matmul starts sooner; later quarters' slices
            # are emitted lazily (inside the quarter) so the nt=0 LT DMAs
            # issue ahead of them on the sync queue.
            emit_s_dma(qoff[0], offs[4])
            # the first LT block of (nt=0, qt=0) right behind it, ahead of
            # the rest of S, so the first chain matmuls unblock early
            lt00 = lt_pool.tile([128, EBLK, 512], FP8, name="lt00", tag="lt")
            nc.sync.dma_start(out=lt00[:], in_=LT[0, 0, :, 0:EBLK * 512])
            nc.sync.dma_start(out=na_sb[:, NKC // 2:, :],
                              in_=naT[:, NKC // 2 * P:])
            emit_s_dma(offs[4], offs[ECQ // 2])
            emit_s_dma(offs[ECQ // 2], qoff[1])

            bias_tiles = {}

            def bias_ap(v):
                v = float(v)
                if v not in bias_tiles:
                    t = const_pool.tile([128, 1], F32,
                                        name=f"bias{len(bias_tiles)}",
                                        tag=f"bias{len(bias_tiles)}")
                    nc.any.memset(t[:], v)
                    bias_tiles[v] = t
                return bias_tiles[v][:]

            a_tiles = [a_pool.tile([128, ECHUNKS, P], FP8, name=f"a{q}",
                                   tag=f"a{q}") for q in range(npair)]

            def emit_phase1_ecs(qt, ci_lo, ci_hi):
                if ci_lo == 0 and qt + 1 < QTRS:
                    emit_s_dma(qoff[qt + 1], qoff[qt + 2])
                for ci in range(ci_lo, ci_hi):
                    ec = qt * ECQ + ci
                    mms = plan[ec]
                    tp = tp_pool.tile([128, P], F32, tag="tp")
                    for j, (pA, pB) in enumerate(mms):
                        d = pB - pA
                        nc.tensor.matmul(
                            tp,
                            lhsT=s_sb[:, offs[ec] + 2 * j:
                                      offs[ec] + 2 * j + 2, :],
                            rhs=na_sb[:, pA:pB + 1:d, :],
                            start=(j == 0), stop=(j == len(mms) - 1),
                            perf_mode=DR)
                    # A masks: split across ACT/DVE; ~20/12 per 32 ops keeps
                    # both engines under the 16.6us/quarter PE chain budget.
                    for q, (tv, _lv) in enumerate(pairs):
                        if _act_able(tv, tmax):
                            eng = ("act"
                                   if (q == 0 and ci % 8 != 7) or ci % 8 < 2
                                   else "dve")
                        else:
                            eng = "dve"
                        _mask_op(nc, a_tiles[q][:, ec, :], tp, tv, tmax,
                                 eng, bias_ap)

            for nt in range(NTILES):
                eps = [chain_pool.tile([128, 512], F32, name=f"ep{_m}",
                                       tag="chain")
                       for _m in range(MTILES)]
                for qt in range(QTRS):
                    if nt == 0:
                        emit_phase1_ecs(qt, 0, ECQ)
                    bq = [b_pool.tile([128, ECQ, 512], FP8, name=f"bq{_q}",
                                      tag="b")
                          for _q in range(npair)]
                    for bb in range(EBQ):
                        if nt == 0 and qt == 0 and bb == 0:
                            lt = lt00
                        else:
                            lt = lt_pool.tile([128, EBLK, 512], FP8,
                                              tag="lt")
                            nc.sync.dma_start(
                                out=lt[:],
                                in_=LT[nt, qt, :,
                                       bb * EBLK * 512:(bb + 1) * EBLK * 512])
                        # the first granule of a quarter is split finer so
                        # the quarter's first chain matmuls unblock sooner
                        gran = ([(0, 2), (2, EBLK // 2), (EBLK // 2, EBLK)]
                                if bb == 0 else [(0, EBLK)])
                        for q, (_tv, lv) in enumerate(pairs):
                            # wave 0 is mask-supply-bound: hand the idle
                            # GPSIMD one big SBUF->SBUF granule per quarter
                            beng = ("gps" if nt == 0 and bb == 1 and q == 1
                                    else "dve")
                            for g0, g1 in gran:
                                _mask_op(
                                    nc,
                                    bq[q][:, bb * EBLK + g0:bb * EBLK + g1,
                                          :],
                                    lt[:, g0:g1, :], lv, lmax, beng,
                                    bias_ap)
                    # accumulation chains, 16 MMs per m-tile this quarter
                    for m in range(MTILES):
                        for kk in range(ECQ // 2):
                            ec2 = qt * ECQ + 2 * kk
                            for q in range(npair):
                                first = (qt == 0 and kk == 0 and q == 0)
                                last = (qt == QTRS - 1
                                        and kk == ECQ // 2 - 1
                                        and q == npair - 1)
                                nc.tensor.matmul(
                                    eps[m],
                                    lhsT=a_tiles[q][:, ec2:ec2 + 2,
                                                    m * 128:(m + 1) * 128],
                                    rhs=bq[q][:, 2 * kk:2 * kk + 2, :],
                                    start=first, stop=last, perf_mode=DR)
                for m in range(MTILES):
                    ot = out_pool.tile([128, 512], F32, tag="out")
                    nc.scalar.copy(out=ot[:], in_=eps[m][:])
                    nc.sync.dma_start(
                        out=en[m * 128:(m + 1) * 128,
                               nt * 512:(nt + 1) * 512],
                        in_=ot[:])
    if not nc.is_finalized():
        nc.finalize()
    return nc


def _get_nc(pairs, plan_key):
    key = (tuple(pairs), plan_key)
    if key not in _nc_cache:
        _nc_cache[key] = _build_nc(tuple(pairs), plan_key)
    return _nc_cache[key]


def _edge_plan(idx):
    """Permute edges so each 128-edge chunk touches few node blocks.

    Full 128-edge runs of each (block, block) bucket become pure chunks
    (1 matmul); bucket remainders are packed into mixed chunks sorted so
    neighbours share their low block (usually 2 matmuls).

    Returns (perm, plan) where plan[c] = tuple of (pA, pB) node-block
    pairs (pA < pB) whose 256 rows feed chunk c's DR matmuls.
    """
    b0 = idx[:, 0] // 128
    b1 = idx[:, 1] // 128
    lo = np.minimum(b0, b1)
    hi = np.maximum(b0, b1)
    key = (lo * NKC + hi).astype(np.int64)
    order = np.argsort(key, kind="stable")
    buckets: dict = {}
    for e in order.tolist():
        buckets.setdefault(int(key[e]), []).append(e)
    pure = []       # (edges128, blocks)
    leftovers = []  # (blocks, edges)
    for k in sorted(buckets):
        edges = buckets[k]
        p, q = divmod(k, NKC)
        blocks = [p] if p == q else [p, q]
        i = 0
        while len(edges) - i >= 128:
            pure.append((edges[i:i + 128], blocks))
            i += 128
        if i < len(edges):
            leftovers.append((blocks, edges[i:]))
    leftovers.sort(key=lambda t: (t[0][0], t[0][-1]))
    mixed = []
    cur_e: list = []
    cur_b: set = set()
    for blocks, edges in leftovers:
        pos = 0
        while pos < len(edges):
            take = min(128 - len(cur_e), len(edges) - pos)
            cur_e.extend(edges[pos:pos + take])
            cur_b.update(blocks)
            pos += take
            if len(cur_e) == 128:
                mixed.append((cur_e, sorted(cur_b)))
                cur_e, cur_b = [], set()
    assert not cur_e and len(pure) + len(mixed) == ECHUNKS
    chunks = pure + mixed
    perm = np.array([e for ch, _b in chunks for e in ch], dtype=np.int64)
    plan = []
    for _ch, blocks in chunks:
        blocks = sorted(set(blocks))
        mms = []
        i = 0
        while i + 1 < len(blocks):
            mms.append((blocks[i], blocks[i + 1]))
            i += 2
        if i < len(blocks):
            b = blocks[i]
            # partner must be a block this chunk does NOT use: its S rows
            # are all-zero for these edges, so the DR matmul adds nothing.
            partner = next(x for x in range(NKC) if x not in blocks)
            mms.append((min(b, partner), max(b, partner)))
        plan.append(tuple(mms))
    return perm, tuple(plan)


def _prep_inputs(node_activations, learned_edge_states, edge_endnode_idx,
                 perm, plan):
    na = np.asarray(node_activations)
    L = np.asarray(learned_edge_states, dtype=np.float32)
    idx = np.asarray(edge_endnode_idx)[perm]

    # S over permuted edges
    Sm = np.zeros((N_NODES, N_EDGES), dtype=np.int16)
    e = np.arange(N_EDGES)
    np.add.at(Sm, (idx[:, 0], e), 2)
    np.add.at(Sm, (idx[:, 1], e), 1)

    tot2 = sum(2 * len(mms) for mms in plan)
    S8 = np.zeros((128, tot2, 128), dtype=NP_FP8)
    off = 0
    for c, mms in enumerate(plan):
        esl = slice(c * 128, (c + 1) * 128)
        for (pA, pB) in mms:
            S8[:, off, :] = Sm[pA * 128:(pA + 1) * 128, esl]
            S8[:, off + 1, :] = Sm[pB * 128:(pB + 1) * 128, esl]
            off += 2
    S8 = S8.reshape(128, tot2 * 128)

    naT = np.ascontiguousarray(na.T).astype(NP_FP8)         # [nodes, pts]
    LTf = np.ascontiguousarray(L.T[perm])                   # [edges, cmp]

    in_maps = []
    for pg in range(PGROUPS):
        for cg in range(CGROUPS):
            # naT tiled: [ki, ko*P+p]
            nat = np.ascontiguousarray(
                naT[:, pg * P:(pg + 1) * P]
                .reshape(NKC, 128, P).transpose(1, 0, 2)
                .reshape(128, NKC * P))
            # LT tiled: [nt, qt, ki, c*512+j]
            lt = np.ascontiguousarray(
                LTf[:, cg * C:(cg + 1) * C]
                .reshape(QTRS, ECQ, 128, NTILES, 512)
                .transpose(3, 0, 2, 1, 4)
                .reshape(NTILES, QTRS, 128, ECQ * 512)).astype(NP_FP8)
            in_maps.append({
                "naT": nat,
                "S": S8,
                "LT": lt,
            })
    return in_maps


def _kept_pairs(edge_type_filter):
    seen = []
    for v in np.asarray(edge_type_filter).ravel().tolist():
        v = int(v)
        if v in _CODE2TEMP and v not in [p[1] for p in seen]:
            seen.append((_CODE2TEMP[v], v))
    return tuple(seen)


def kernel(node_activations, learned_edge_states, edge_endnode_idx,
           edge_type_filter, _trace=False, _tmpdir=None):
    pairs = _kept_pairs(edge_type_filter)
    L = np.asarray(learned_edge_states, dtype=np.float32)
    null_count = (L == 0.0).sum(axis=1).astype(np.float32)   # [n_cmp]
    if len(pairs) == 0:
        # nothing kept: energies are null_count rows broadcast
        en = np.broadcast_to(null_count[None, :], (N_PTS, N_CMP)).copy()
        return en - en.min()

    perm, plan = _edge_plan(np.asarray(edge_endnode_idx))
    nc = _get_nc(pairs, plan)
    in_maps = _prep_inputs(node_activations, learned_edge_states,
                           edge_endnode_idx, perm, plan)
    res = run_bass_kernel_spmd(nc, in_maps, core_ids=list(range(8)),
                               trace=_trace, tmpdir=_tmpdir)
    out = np.empty((N_PTS, N_CMP), dtype=np.float32)
    for ci in range(8):
        pg, cg = ci // CGROUPS, ci % CGROUPS
        out[pg * P:(pg + 1) * P, cg * C:(cg + 1) * C] = res.results[ci]["en"]
    out += null_count[None, :]
    out -= out.min()
    if _trace:
        kernel._last_results = res
    return out


# revision 27
# speedup vs baseline: 1.8843x; 1.8843x over previous
"""HNet energy-via-edge-matching kernel for 8 Trainium2 NeuronCores.

Math (matches the reference exactly, in exact integer arithmetic):
  temp[i,e] = 2*na[i, idx0[e]] + na[i, idx1[e]]          in {0,1,2,3}
  es = code[temp], code = [NOR=2, NCONV=3, NIMPL=5, AND=9]
  filter keeps es values in edge_type_filter, else NULL=0
  energies[i,j] = #{e: L[j,e]==es'[i,e] or L[j,e]==0}
               = null_count[j] + sum_{v kept} (temp==tmap[v]) . (L==v)
  output = energies - min(energies)

Device decomposition per core (4 point-groups x 2 cmp-groups):
  phase 1: tT[e,i] = sum_n S[n,e]*naT[n,i], S = 2*onehot(idx0)+onehot(idx1)
           -> fp8 DoubleRow matmuls.  Edges are permuted on the host so
           each 128-edge chunk touches ~2 node blocks (sorted by the
           unordered node-block pair of its endpoints); each chunk then
           needs only ceil(nblocks/2) DR matmuls instead of 4.
           A_v[e,i] = (tT==tmap[v]) masks (fp8), split across ACT/DVE.
  phase 2: per cmp tile nt (512 cols): B_v[e,j] = (LT==v) masks (fp8),
           energies[i,j] = sum_e A_v^T B_v via fp8 DoubleRow matmuls.
           Emission is pipelined at e-quarter granularity: phase-1 work
           for quarter k is emitted inside the nt=0 wave so the PE never
           waits for the full mask set (chains mid-stall on range deps).
Host only: input staging/layout (incl. the edge permutation), null_count
  row add, global min subtract during unshard (exact fp32 integer math).
"""

import numpy as np
import ml_dtypes

import concourse.bacc as bacc
import concourse.mybir as mybir
from concourse.tile import TileContext
from concourse.bass_utils import run_bass_kernel_spmd

# ---- problem constants (hardcoded from spec) ----
N_PTS, N_NODES, N_EDGES, N_CMP = 2048, 1024, 8192, 4096
PGROUPS, CGROUPS = 4, 2          # 8 cores = 4 point-groups x 2 cmp-groups
P = N_PTS // PGROUPS             # 512 points per core
C = N_CMP // CGROUPS             # 2048 cmp columns per core
ECHUNKS = N_EDGES // 128         # 64 edge chunks of 128
NKC = N_NODES // 128             # 8 node chunks of 128
NTILES = C // 512                # 4 cmp tiles of 512 per core
MTILES = P // 128                # 4 point chunks of 128 per core
QTRS = 4                         # e-quarters for pipelined emission
ECQ = ECHUNKS // QTRS            # 16 edge chunks per quarter
EBLK = 8                         # edge chunks per LT DMA block
EBQ = ECQ // EBLK                # LT blocks per quarter (2)

FP8 = mybir.dt.float8e4
F32 = mybir.dt.float32
NP_FP8 = ml_dtypes.float8_e4m3
DR = mybir.MatmulPerfMode.DoubleRow
EQ = mybir.AluOpType.is_equal
RELU = mybir.ActivationFunctionType.Relu

_CODE2TEMP = {2: 0, 3: 1, 5: 2, 9: 3}   # EDG code value -> temp index

_nc_cache: dict = {}


def _act_able(value, alphabet_max):
    return value == alphabet_max or value == 0


def _mask_op(nc, out, in_, value, alphabet_max, engine, bias_ap):
    """Emit out = (in_ == value) as {0.0, 1.0} fp8.

    "act" uses an exact one-relu indicator (valid when value is the
    alphabet max: relu(x-(value-1)); or value==0: relu(1-x)).
    "dve" uses is_equal.  Exact on these small-integer alphabets.
    """
    if engine == "act":
        if value == alphabet_max and value != 0:
            nc.scalar.activation(out, in_, RELU, bias=bias_ap(1 - value),
                                 scale=1.0)
        elif value == 0:
            nc.scalar.activation(out, in_, RELU, bias=bias_ap(1), scale=-1.0)
        else:
            raise ValueError(f"no act indicator for {value}")
        return
    nc.vector.tensor_scalar(out=out, in0=in_, scalar1=float(value),
                            scalar2=None, op0=EQ)


def _build_nc(pairs, plan_key):
    """Build the SPMD Bass program.

    pairs = tuple of (temp_val, L_val).
    plan_key = tuple per edge chunk of ((pA, pB), ...) block pairs.
    """
    nc = bacc.Bacc(None)
    plan = plan_key
    offs = []            # S slot offset per chunk
    tot2 = 0
    for mms in plan:
        offs.append(tot2)
        tot2 += 2 * len(mms)
    # quarter slice boundaries in S slots
    qoff = [offs[q * ECQ] for q in range(QTRS)] + [tot2]

    npair = len(pairs)
    tmax = max((tv for tv, _ in pairs), default=0)
    lmax = 9  # EDG alphabet max
    # pre-tiled inputs (host lays out so every DMA is per-partition dense):
    #   naT : [128, NKC*P]            [ki, ko*P+p]   = na[pg*P+p, ko*128+ki]
    #   S   : [128, TOT2*128]  slot s=(off_c+2j+h): [ki, s*128+el] =
    #           S[blk(c,j,h)*128+ki, perm_e(c*128+el)]
    #   LT  : [NTILES, QTRS, 128, ECQ*512] [nt,qt,ki, c*512+j] =
    #           L[cg*C+nt*512+j, ((qt*ECQ+c)*128+ki th permuted edge)]
    naT = nc.dram_tensor("naT", [128, NKC * P], FP8, kind="ExternalInput")
    S = nc.dram_tensor("S", [128, tot2 * 128], FP8, kind="ExternalInput")
    LT = nc.dram_tensor("LT", [NTILES, QTRS, 128, ECQ * 512], FP8,
                        kind="ExternalInput")
    en = nc.dram_tensor("en", [P, C], F32, kind="ExternalOutput")

    with TileContext(nc) as tc:
        with (
            tc.tile_pool(name="const", bufs=1) as const_pool,
            tc.tile_pool(name="amask", bufs=1) as a_pool,
            tc.tile_pool(name="bmask", bufs=3 * npair + 1) as b_pool,
            tc.tile_pool(name="lt", bufs=5) as lt_pool,
            tc.tile_pool(name="out", bufs=4) as out_pool,
            tc.tile_pool(name="chain", bufs=6, space="PSUM") as chain_pool,
            tc.tile_pool(name="tp", bufs=2, space="PSUM") as tp_pool,
        ):
            na_sb = const_pool.tile([128, NKC, P], FP8, tag="na")
            # low node blocks first: the first (key-sorted) phase-1 chunks
            # only touch them, so the first matmul isn't gated on all of na
            nc.sync.dma_start(out=na_sb[:, :NKC // 2, :],
                              in_=naT[:, :NKC // 2 * P])
            s_sb = const_pool.tile([128, tot2, 128], FP8, tag="s")

            def emit_s_dma(lo, hi):
                if hi > lo:
                    nc.sync.dma_start(
                        out=s_sb[:, lo:hi, :],
                        in_=S[:, lo * 128:hi * 128])

            # First quarter's S arrives in three slices up front so the very
            # first phase-1 matmul starts sooner; later quarters' slices
            # are emitted lazily (inside the quarter) so the nt=0 LT DMAs
            # issue ahead of them on the sync queue.
            emit_s_dma(qoff[0], offs[4])
            # the first LT block of (nt=0, qt=0) right behind it, ahead of
            # the rest of S, so the first chain matmuls unblock early
            lt00 = lt_pool.tile([128, EBLK, 512], FP8, name="lt00", tag="lt")
            nc.sync.dma_start(out=lt00[:], in_=LT[0, 0, :, 0:EBLK * 512])
            nc.sync.dma_start(out=na_sb[:, NKC // 2:, :],
                              in_=naT[:, NKC // 2 * P:])
            emit_s_dma(offs[4], offs[ECQ // 2])
            emit_s_dma(offs[ECQ // 2], qoff[1])

            bias_tiles = {}

            def bias_ap(v):
                v = float(v)
                if v not in bias_tiles:
                    t = const_pool.tile([128, 1], F32,
                                        name=f"bias{len(bias_tiles)}",
                                        tag=f"bias{len(bias_tiles)}")
                    nc.any.memset(t[:], v)
                    bias_tiles[v] = t
                return bias_tiles[v][:]

            a_tiles = [a_pool.tile([128, ECHUNKS, P], FP8, name=f"a{q}",
                                   tag=f"a{q}") for q in range(npair)]

            def emit_phase1_ecs(qt, ci_lo, ci_hi):
                if ci_lo == 0 and qt + 1 < QTRS:
                    emit_s_dma(qoff[qt + 1], qoff[qt + 2])
                for ci in range(ci_lo, ci_hi):
                    ec = qt * ECQ + ci
                    mms = plan[ec]
                    tp = tp_pool.tile([128, P], F32, tag="tp")
                    for j, (pA, pB) in enumerate(mms):
                        d = pB - pA
                        nc.tensor.matmul(
                            tp,
                            lhsT=s_sb[:, offs[ec] + 2 * j:
                                      offs[ec] + 2 * j + 2, :],
                            rhs=na_sb[:, pA:pB + 1:d, :],
                            start=(j == 0), stop=(j == len(mms) - 1),
                            perf_mode=DR)
                    # A masks: split across ACT/DVE; ~20/12 per 32 ops keeps
                    # both engines under the 16.6us/quarter PE chain budget.
                    for q, (tv, _lv) in enumerate(pairs):
                        if _act_able(tv, tmax):
                            eng = "act" if (q == 0 or ci % 8 < 2) else "dve"
                        else:
                            eng = "dve"
                        _mask_op(nc, a_tiles[q][:, ec, :], tp, tv, tmax,
                                 eng, bias_ap)

            for nt in range(NTILES):
                eps = [chain_pool.tile([128, 512], F32, name=f"ep{_m}",
                                       tag="chain")
                       for _m in range(MTILES)]
                for qt in range(QTRS):
                    if nt == 0:
                        emit_phase1_ecs(qt, 0, ECQ)
                    bq = [b_pool.tile([128, ECQ, 512], FP8, name=f"bq{_q}",
                                      tag="b")
                          for _q in range(npair)]
                    for bb in range(EBQ):
                        if nt == 0 and qt == 0 and bb == 0:
                            lt = lt00
                        else:
                            lt = lt_pool.tile([128, EBLK, 512], FP8,
                                              tag="lt")
                            nc.sync.dma_start(
                                out=lt[:],
                                in_=LT[nt, qt, :,
                                       bb * EBLK * 512:(bb + 1) * EBLK * 512])
                        # the first granule of a quarter is split finer so
                        # the quarter's first chain matmuls unblock sooner
                        gran = ([(0, 2), (2, EBLK // 2), (EBLK // 2, EBLK)]
                                if bb == 0 else [(0, EBLK)])
                        for q, (_tv, lv) in enumerate(pairs):
                            for g0, g1 in gran:
                                _mask_op(
                                    nc,
                                    bq[q][:, bb * EBLK + g0:bb * EBLK + g1,
                                          :],
                                    lt[:, g0:g1, :], lv, lmax, "dve",
                                    bias_ap)
                    # accumulation chains, 16 MMs per m-tile this quarter
                    for m in range(MTILES):
                        for kk in range(ECQ // 2):
                            ec2 = qt * ECQ + 2 * kk
                            for q in range(npair):
                                first = (qt == 0 and kk == 0 and q == 0)
                                last = (qt == QTRS - 1
                                        and kk == ECQ // 2 - 1
                                        and q == npair - 1)
                                nc.tensor.matmul(
                                    eps[m],
                                    lhsT=a_tiles[q][:, ec2:ec2 + 2,
                                                    m * 128:(m + 1) * 128],
                                    rhs=bq[q][:, 2 * kk:2 * kk + 2, :],
                                    start=first, stop=last, perf_mode=DR)
                for m in range(MTILES):
                    ot = out_pool.tile([128, 512], F32, tag="out")
                    nc.scalar.copy(out=ot[:], in_=eps[m][:])
                    nc.sync.dma_start(
                        out=en[m * 128:(m + 1) * 128,
                               nt * 512:(nt + 1) * 512],
                        in_=ot[:])
    if not nc.is_finalized():
        nc.finalize()
    return nc


def _get_nc(pairs, plan_key):
    key = (tuple(pairs), plan_key)
    if key not in _nc_cache:
        _nc_cache[key] = _build_nc(tuple(pairs), plan_key)
    return _nc_cache[key]


def _edge_plan(idx):
    """Permute edges so each 128-edge chunk touches few node blocks.

    Full 128-edge runs of each (block, block) bucket become pure chunks
    (1 matmul); bucket remainders are packed into mixed chunks sorted so
    neighbours share their low block (usually 2 matmuls).

    Returns (perm, plan) where plan[c] = tuple of (pA, pB) node-block
    pairs (pA < pB) whose 256 rows feed chunk c's DR matmuls.
    """
    b0 = idx[:, 0] // 128
    b1 = idx[:, 1] // 128
    lo = np.minimum(b0, b1)
    hi = np.maximum(b0, b1)
    key = (lo * NKC + hi).astype(np.int64)
    order = np.argsort(key, kind="stable")
    buckets: dict = {}
    for e in order.tolist():
        buckets.setdefault(int(key[e]), []).append(e)
    pure = []       # (edges128, blocks)
    leftovers = []  # (blocks, edges)
    for k in sorted(buckets):
        edges = buckets[k]
        p, q = divmod(k, NKC)
        blocks = [p] if p == q else [p, q]
        i = 0
        while len(edges) - i >= 128:
            pure.append((edges[i:i + 128], blocks))
            i += 128
        if i < len(edges):
            leftovers.append((blocks, edges[i:]))
    leftovers.sort(key=lambda t: (t[0][0], t[0][-1]))
    mixed = []
    cur_e: list = []
    cur_b: set = set()
    for blocks, edges in leftovers:
        pos = 0
        while pos < len(edges):
            take = min(128 - len(cur_e), len(edges) - pos)
            cur_e.extend(edges[pos:pos + take])
            cur_b.update(blocks)
            pos += take
            if len(cur_e) == 128:
                mixed.append((cur_e, sorted(cur_b)))
                cur_e, cur_b = [], set()
    assert not cur_e and len(pure) + len(mixed) == ECHUNKS
    chunks = pure + mixed
    perm = np.array([e for ch, _b in chunks for e in ch], dtype=np.int64)
    plan = []
    for _ch, blocks in chunks:
        blocks = sorted(set(blocks))
        mms = []
        i = 0
        while i + 1 < len(blocks):
            mms.append((blocks[i], blocks[i + 1]))
            i += 2
        if i < len(blocks):
            b = blocks[i]
            # partner must be a block this chunk does NOT use: its S rows
            # are all-zero for these edges, so the DR matmul adds nothing.
            partner = next(x for x in range(NKC) if x not in blocks)
            mms.append((min(b, partner), max(b, partner)))
        plan.append(tuple(mms))
    return perm, tuple(plan)


def _prep_inputs(node_activations, learned_edge_states, edge_endnode_idx,
                 perm, plan):
    na = np.asarray(node_activations)
    L = np.asarray(learned_edge_states, dtype=np.float32)
    idx = np.asarray(edge_endnode_idx)[perm]

    # S over permuted edges
    Sm = np.zeros((N_NODES, N_EDGES), dtype=np.int16)
    e = np.arange(N_EDGES)
    np.add.at(Sm, (idx[:, 0], e), 2)
    np.add.at(Sm, (idx[:, 1], e), 1)

    tot2 = sum(2 * len(mms) for mms in plan)
    S8 = np.zeros((128, tot2, 128), dtype=NP_FP8)
    off = 0
    for c, mms in enumerate(plan):
        esl = slice(c * 128, (c + 1) * 128)
        for (pA, pB) in mms:
            S8[:, off, :] = Sm[pA * 128:(pA + 1) * 128, esl]
            S8[:, off + 1, :] = Sm[pB * 128:(pB + 1) * 128, esl]
            off += 2
    S8 = S8.reshape(128, tot2 * 128)

    naT = np.ascontiguousarray(na.T).astype(NP_FP8)         # [nodes, pts]
    LTf = np.ascontiguousarray(L.T[perm])                   # [edges, cmp]

    in_maps = []
    for pg in range(PGROUPS):
        for cg in range(CGROUPS):
            # naT tiled: [ki, ko*P+p]
            nat = np.ascontiguousarray(
                naT[:, pg * P:(pg + 1) * P]
                .reshape(NKC, 128, P).transpose(1, 0, 2)
                .reshape(128, NKC * P))
            # LT tiled: [nt, qt, ki, c*512+j]
            lt = np.ascontiguousarray(
                LTf[:, cg * C:(cg + 1) * C]
                .reshape(QTRS, ECQ, 128, NTILES, 512)
                .transpose(3, 0, 2, 1, 4)
                .reshape(NTILES, QTRS, 128, ECQ * 512)).astype(NP_FP8)
            in_maps.append({
                "naT": nat,
                "S": S8,
                "LT": lt,
            })
    return in_maps


def _kept_pairs(edge_type_filter):
    seen = []
    for v in np.asarray(edge_type_filter).ravel().tolist():
        v = int(v)
        if v in _CODE2TEMP and v not in [p[1] for p in seen]:
            seen.append((_CODE2TEMP[v], v))
    return tuple(seen)


def kernel(node_activations, learned_edge_states, edge_endnode_idx,
           edge_type_filter, _trace=False, _tmpdir=None):
    pairs = _kept_pairs(edge_type_filter)
    L = np.asarray(learned_edge_states, dtype=np.float32)
    null_count = (L == 0.0).sum(axis=1).astype(np.float32)   # [n_cmp]
    if len(pairs) == 0:
        # nothing kept: energies are null_count rows broadcast
        en = np.broadcast_to(null_count[None, :], (N_PTS, N_CMP)).copy()
        return en - en.min()

    perm, plan = _edge_plan(np.asarray(edge_endnode_idx))
    nc = _get_nc(pairs, plan)
    in_maps = _prep_inputs(node_activations, learned_edge_states,
                           edge_endnode_idx, perm, plan)
    res = run_bass_kernel_spmd(nc, in_maps, core_ids=list(range(8)),
                               trace=_trace, tmpdir=_tmpdir)
    out = np.empty((N_PTS, N_CMP), dtype=np.float32)
    for ci in range(8):
        pg, cg = ci // CGROUPS, ci % CGROUPS
        out[pg * P:(pg + 1) * P, cg * C:(cg + 1) * C] = res.results[ci]["en"]
    out += null_count[None, :]
    out -= out.min()
    if _trace:
        kernel._last_results = res
    return out
